# revision 14
# baseline (speedup 1.0000x reference)
"""Trainium2 Bass kernel: masked-bilinear channel-mixing Conv2d (ring-pair form).

reference math (N=4, C=96, H=W=32, O=96, K=3, PAD=1):
    p = avgpool3x3(x, count_include_pad) -> [N, C, H, W] -> [N, L=1024, C]
    wm = weight * mask                              [O, C, C]
    y[n,l,o] = sum_{c,d} wm[o,c,d] p[n,l,c] p[n,l,d] + bias[o]

The mask has a zero diagonal, so only c!=d pairs contribute. Writing
d = (c+k) mod 96, every ordered pair has a unique ring offset k in 1..95,
and offsets k and 96-k cover the same unordered pairs. Folding:

    y[o,l]   = sum_{k=1..48} sum_c WM2_k[c,o] * r_k[c,l]
    r_k[c,l] = p[c,l] * p[(c+k)%96, l]
    WM2_k    = A_k*maskA_k + B_k*maskB_k   (B empty for k=48)
    A_k[c,o] = weight[o, c, (c+k)%96],  B_k[c,o] = weight[o, (c+k)%96, c]

vs. the two-pass einsum dataflow this cuts PE work ~4x and halves the
elementwise middle stage. All DMA queues share one ~250GB/s channel, so
the partition-rotated copies rot_k are generated ON-CHIP instead of
streamed: rot_k = (I2[:, k:k+96])^T @ p on the PE (I2 is a [96,192]
wrapped identity, so the free-dim slice is a rotation matrix), drained
PSUM->SBUF(bf16) in 3-bank batches by the otherwise-idle ACT engine.

Sharding: data-parallel over locations -> 8 cores, each takes half of one
image (16 rows = 512 locations), all 96 output channels.

Per-core pipeline (engines in parallel):
  PE:   warmup burst (HAM clock gate), 48 rotation matmuls + 48
        accumulating GEMMs into one PSUM bank, interleaved per k-group.
  DVE:  pooling (f32, x split in halves) -> pt16 (x1/9, bf16) -> pd12 ->
        per weight block: w*m multiply + A+B fold -> r products
        (bf16 TT 2x) chasing the ACT drains.
  ACT:  rot PSUM->SBUF drains; bias add on the final y copy.
  DMA:  x halves + I2 + y on sync; weights in 4 [w|m] blocks on scalar.
"""
import numpy as np

import concourse.bass as bass
import concourse.bacc as bacc
import concourse.mybir as mybir
from concourse import tile
from concourse import bass_utils

C = 96
O = 96
HS = 16           # rows per core shard
W = 32
L = HS * W        # 512 locations per core
N_CORES = 8
NK = 48           # ring offsets
NBLK = 4          # weight DMA blocks (12 k's each)
KPB = 12          # k's per block
BLKW = 2 * KPB * O   # weight cols per block half (A,B interleaved: 2304)
NK_PE = 33        # k=1..NK_PE rotated on-chip (PE matmul + ACT drain)
ROT_GROUPS = (3, 9, 12, 9, 9, 6)    # r-op k-groups (PE part multiples of 3)
WARMUP_MMS = 8
F32 = mybir.dt.float32
BF16 = mybir.dt.bfloat16


def _build_kernel(nc: bass.Bass):
    xs_d = nc.dram_tensor("xs", [C, 18 * 34], F32, kind="ExternalInput")
    i2_d = nc.dram_tensor("i2", [C, 2 * C], BF16, kind="ExternalInput")
    wcat_d = nc.dram_tensor("wcat", [C, NBLK * 2 * BLKW], BF16,
                            kind="ExternalInput")
    b_d = nc.dram_tensor("bias", [O, 1], F32, kind="ExternalInput")
    y_d = nc.dram_tensor("y", [O, L], F32, kind="ExternalOutput")

    with tile.TileContext(nc) as tc:
        with (
            tc.tile_pool(name="const", bufs=1) as cpool,
            tc.tile_pool(name="dram", bufs=1, space="DRAM") as dpool,
            tc.tile_pool(name="rps", bufs=2, space="PSUM") as rpsum,
            tc.tile_pool(name="wps", bufs=1, space="PSUM") as wpsum,
            tc.tile_pool(name="yps", bufs=1, space="PSUM") as ypsum,
        ):
            # ---- PE warmup ASAP (garbage matmuls on a scratch bank) ----
            warm16 = cpool.tile([C, L], BF16)
            nc.vector.memset(warm16[:], 0.0)
            wps = wpsum.tile([C, L], F32)
            for _ in range(WARMUP_MMS):
                nc.tensor.matmul(wps[:], warm16[:, 0:C], warm16[:],
                                 start=True, stop=True, skip_group_check=True)

            # ---- input DMAs ----
            # x halves FIRST on the scalar queue: the DMA channel is a
            # single ~250GB/s resource ordered by issue, so x must not
            # queue behind an 875KB weight block.
            xs = cpool.tile([C, 18 * 34], F32)
            i2 = cpool.tile([C, 2 * C], BF16)
            bias = cpool.tile([O, 1], F32)
            HHALF = 9 * 34
            nc.scalar.dma_start(xs[:, 0:HHALF], xs_d.ap()[:, 0:HHALF])
            nc.scalar.dma_start(xs[:, HHALF:2 * HHALF],
                                xs_d.ap()[:, HHALF:2 * HHALF])
            nc.sync.dma_start(i2[:], i2_d.ap())
            nc.sync.dma_start(bias[:], b_d.ap())
            wcat = cpool.tile([C, NBLK * 2 * BLKW], BF16)
            for g in range(NBLK):
                base = g * 2 * BLKW
                nc.scalar.dma_start(wcat[:, base:base + 2 * BLKW],
                                    wcat_d.ap()[:, base:base + 2 * BLKW])

            # ---- pooling: horizontal (per x half) then vertical 3-tap ----
            # pinned high priority so the scheduler cannot interleave the
            # weight multiplies into the pool->pt16->pd12 chain
            s1 = cpool.tile([C, 18 * 32], F32)
            s2 = cpool.tile([C, 18 * 32], F32)
            pt_raw = cpool.tile([C, L], F32)
            pt2 = cpool.tile([C, L], F32)
            pt16 = cpool.tile([C, L], BF16)
            pd12 = cpool.tile([C, 12 * L], BF16)
            with tc.high_priority():
                x3 = xs[:].rearrange("c (h w) -> c h w", h=18)
                s1v = s1[:].rearrange("c (h w) -> c h w", h=18)
                s2v = s2[:].rearrange("c (h w) -> c h w", h=18)
                for h0, h1 in ((0, 9), (9, 18)):
                    nc.vector.tensor_add(s1v[:, h0:h1, :],
                                         x3[:, h0:h1, 0:32],
                                         x3[:, h0:h1, 1:33])
                    nc.vector.tensor_add(s2v[:, h0:h1, :], s1v[:, h0:h1, :],
                                         x3[:, h0:h1, 2:34])
                ptv = pt_raw[:].rearrange("c (h w) -> c h w", h=HS)
                pt2v = pt2[:].rearrange("c (h w) -> c h w", h=HS)
                nc.vector.tensor_add(pt2v, s2v[:, 0:16, :], s2v[:, 1:17, :])
                nc.vector.tensor_add(ptv, pt2v, s2v[:, 2:18, :])
                # p = boxsum/9; p enters the quadratic form twice -> 1/81
                nc.vector.tensor_scalar_mul(pt16[:], pt_raw[:], 1.0 / 9.0)
                # pd12 = p repeated 12x along free (in0 for every r op)
                nc.vector.tensor_copy(
                    pd12[:].rearrange("c (r l) -> c r l", r=12),
                    pt16[:].unsqueeze(1).broadcast_to((C, 12, L)))

            # ---- stage p to DRAM as [p; p[0:48]]; k>NK_PE rotations come
            # from DRAM (linear APs can walk partitions), offloading ACT
            pp = dpool.tile([C + NK, L], BF16)
            nc.sync.dma_start(pp[0:C, :], pt16[:])
            nc.sync.dma_start(pp[C:C + NK, :], pt16[0:NK, :])

            rot = cpool.tile([C, NK * L], BF16)
            r = cpool.tile([C, NK * L], BF16)
            wmf = cpool.tile([C, NK * O], BF16)    # folded A+B weights
            y_ps = ypsum.tile([O, L], F32)
            y_sb = cpool.tile([O, L], F32)

            # DMA-sourced rotations (k > NK_PE), issued now; they queue
            # behind the pp completion and stream during the PE phase.
            # rot[c, k-1+j, l] = pp[c + k + j, l]
            rot3 = rot[:].rearrange("c (j l) -> c j l", j=NK)
            pp_t = pp[:].tensor
            k0d = NK_PE + 1
            for gsz in ROT_GROUPS[4:]:
                src = bass.AP(tensor=pp_t, offset=k0d * L,
                              ap=[[L, C], [L, gsz], [1, L]])
                nc.sync.dma_start(rot3[:, k0d - 1:k0d - 1 + gsz, :], src)
                k0d += gsz

            # rot_k = (I2[:, k:k+96])^T @ p ; drain PSUM->SBUF per 3 k's
            def rot_triple(kt):  # kt = 0..15, covers k = 3*kt+1 .. 3*kt+3
                ps = rpsum.tile([C, 3 * L], F32, tag="rotps")
                for j in range(3):
                    k = 3 * kt + 1 + j
                    nc.tensor.matmul(ps[:, j * L:(j + 1) * L],
                                     i2[:, k:k + C], pt16[:],
                                     start=True, stop=True,
                                     skip_group_check=True)
                nc.scalar.activation(rot[:, 3 * kt * L:(3 * kt + 3) * L],
                                     ps[:],
                                     mybir.ActivationFunctionType.Copy)

            def wm_block(g):
                # w*m for block g (A,B unit-interleaved), then fold A+B
                base = g * 2 * BLKW
                wm2g = cpool.tile([C, BLKW], BF16, tag="wm2")
                nc.vector.tensor_mul(wm2g[:], wcat[:, base:base + BLKW],
                                     wcat[:, base + BLKW:base + 2 * BLKW])
                v = wm2g[:].rearrange("c (j t) -> c j t", j=KPB)
                nc.vector.tensor_add(
                    wmf[:, g * KPB * O:(g + 1) * KPB * O]
                    .rearrange("c (j t) -> c j t", j=KPB),
                    v[:, :, 0:O], v[:, :, O:2 * O])

            gemm_emitted = 0

            def emit_gemm(k_hi):
                nonlocal gemm_emitted
                while gemm_emitted < k_hi:
                    k = gemm_emitted + 1
                    nc.tensor.matmul(
                        y_ps[:], wmf[:, (k - 1) * O:k * O],
                        r[:, (k - 1) * L:k * L],
                        start=(k == 1), stop=(k == NK),
                        skip_group_check=True)
                    gemm_emitted += 1

            # ---- pipelined emission ----
            # DVE: wm_block / r ops in readiness order; PE: rot triples run
            # ahead of the GEMM chain; ACT: drains in k order.
            wm_block(0)
            k0 = 1
            blocks_done = 1
            prev_k = 0
            for gsz in ROT_GROUPS:
                if k0 <= NK_PE:
                    for kt in range((k0 - 1) // 3, (k0 - 1 + gsz) // 3):
                        rot_triple(kt)
                nc.vector.tensor_mul(
                    r[:, (k0 - 1) * L:(k0 - 1 + gsz) * L],
                    pd12[:, 0:gsz * L],
                    rot[:, (k0 - 1) * L:(k0 - 1 + gsz) * L])
                k0 += gsz
                while blocks_done * KPB < min(k0 - 1 + 12, NK) \
                        and blocks_done < NBLK:
                    wm_block(blocks_done)
                    blocks_done += 1
                # GEMM trails the rot triples by one group so its r-wait
                # never blocks queued rotation matmuls in the PE FIFO
                emit_gemm(prev_k)
                prev_k = k0 - 1
            emit_gemm(NK)

            # ---- bias + output ----
            nc.scalar.activation(y_sb[:], y_ps[:],
                                 mybir.ActivationFunctionType.Identity,
                                 bias=bias[:])
            nc.sync.dma_start(y_d.ap(), y_sb[:])

    return nc


_NC_CACHE = {}


def _get_nc():
    if "nc" not in _NC_CACHE:
        nc = bacc.Bacc("TRN2", target_bir_lowering=False, debug=False,
                       enable_asserts=False)
        _build_kernel(nc)
        nc.compile()
        _NC_CACHE["nc"] = nc
    return _NC_CACHE["nc"]


def _to_bf16(a):
    import ml_dtypes
    return np.asarray(a, dtype=ml_dtypes.bfloat16)


def _prep_shards(x, weight, mask, bias):
    xpad = np.pad(np.asarray(x, np.float32), ((0, 0), (0, 0), (1, 1), (1, 1)))
    w = np.asarray(weight, np.float32)
    m = np.asarray(mask, np.float32)
    cs = np.arange(C)
    # I2: wrapped identity, I2[c, j] = 1 iff j % 96 == c
    i2 = np.zeros((C, 2 * C), np.float32)
    i2[cs, cs] = 1.0
    i2[cs, cs + C] = 1.0
    i2 = _to_bf16(i2)
    # block g holds k in [12g+1, 12g+12]; per k: A unit then B unit
    wcat = np.zeros((C, NBLK, 2, KPB, 2, O), np.float32)
    for k in range(1, NK + 1):
        d = (cs + k) % C
        g, j = (k - 1) // KPB, (k - 1) % KPB
        wcat[:, g, 0, j, 0, :] = w[:, cs, d].T
        wcat[:, g, 1, j, 0, :] = m[:, cs, d].T
        if k < NK:
            wcat[:, g, 0, j, 1, :] = w[:, d, cs].T
            wcat[:, g, 1, j, 1, :] = m[:, d, cs].T
    wcat16 = _to_bf16(np.ascontiguousarray(
        wcat.reshape(C, NBLK * 2 * BLKW)))
    b = np.asarray(bias, np.float32).reshape(O, 1)
    in_maps = []
    for core in range(N_CORES):
        n, half = core // 2, core % 2
        h0 = half * HS
        xs = np.ascontiguousarray(
            xpad[n, :, h0:h0 + 18, :].reshape(C, 18 * 34))
        in_maps.append({"xs": xs, "i2": i2, "wcat": wcat16, "bias": b})
    return in_maps


def run_sharded(x, weight, mask, bias, **run_kwargs):
    """Run on the 8 NeuronCores; returns (y_full, BassKernelResults)."""
    nc = _get_nc()
    in_maps = _prep_shards(x, weight, mask, bias)
    res = bass_utils.run_bass_kernel_spmd(
        nc, in_maps, core_ids=list(range(N_CORES)), **run_kwargs)
    n_img = np.asarray(x).shape[0]
    y = np.empty((n_img, O, 32, 32), dtype=np.float32)
    for core in range(N_CORES):
        n, half = core // 2, core % 2
        h0 = half * HS
        y[n, :, h0:h0 + HS, :] = res.results[core]["y"].reshape(O, HS, W)
    return y, res


def kernel(x, weight, mask, bias):
    y, _ = run_sharded(x, weight, mask, bias)
    return y
